# revision 1
# baseline (speedup 1.0000x reference)
"""Trainium2 Bass kernel for nn_Meta_Graph1_40114994545303 (gnn_message_passing).

Math: the reference returns only the global-node row of the GCN output.
With mask = (attribute_label > 0), star adjacency means
    out[s, :] = tanh( (sum_a mask[s,a] * attribute_feat[s,a,:]) @ W + b )
and x never reaches the output (adj[A, A] = 0).

Strategy: data-parallel over batch, 32 samples per core on 8 cores.
Per core:
  stage 1: masked sum over attributes as a block-diagonal matmul
           (feat streamed as the moving operand, mask block-diag stationary)
  transpose the [32, 2048] masked sum to [2048, 32] via DVE 32x32 blocks
  stage 2: [32, 2048] @ W as 16 K-chunk matmuls with the masked-sum
           transposed chunks stationary and W streamed; bias folded in as a
           rank-1 (K=1) matmul of ones x b into the same PSUM accumulation
  tanh on the scalar engine, DMA out.
"""

import os

import numpy as np

import concourse.bacc as bacc
import concourse.mybir as mybir
from concourse.tile import TileContext

B, A, D = 256, 32, 2048
NCORES = 8
S = B // NCORES  # 32 samples per core
P = 128
KC1 = (S * A) // P  # 8 k-chunks in stage 1 (contraction over (sample, attr))
KC2 = D // P  # 16 k-chunks in stage 2 (contraction over d_in)
NT = D // 512  # 4 psum-bank-wide column tiles
F32 = mybir.dt.float32

COMPUTE_DTYPE = os.environ.get("GNN_KERNEL_DTYPE", "fp16")


def build_nc(compute_dtype: str = COMPUTE_DTYPE):
    cdt = {"f32": mybir.dt.float32, "bf16": mybir.dt.bfloat16, "fp16": mybir.dt.float16}[compute_dtype]
    # DMA batching: k-chunks (128 rows x 2048 cols) per dma_start.
    cf = 2 if compute_dtype == "f32" else 4  # feat chunks per DMA
    cw = 4 if compute_dtype == "f32" else 4  # W chunks per DMA
    wbufs = 2 if compute_dtype == "f32" else 4
    nc = bacc.Bacc("TRN2", target_bir_lowering=False, debug=False)

    feat = nc.dram_tensor("feat", [S * A, D], cdt, kind="ExternalInput")
    mbdt = nc.dram_tensor("mbdt", [P, KC1 * S], cdt, kind="ExternalInput")
    w = nc.dram_tensor("w", [D, D], cdt, kind="ExternalInput")
    bias = nc.dram_tensor("bias", [1, D], cdt, kind="ExternalInput")
    out = nc.dram_tensor("out", [S, D], F32, kind="ExternalOutput")

    with TileContext(nc) as tc:
        with (
            tc.tile_pool(name="const", bufs=1) as cpool,
            tc.tile_pool(name="featp", bufs=2) as fpool,
            tc.tile_pool(name="wp", bufs=wbufs) as wpool,
            tc.tile_pool(name="msc", bufs=1) as mcpool,
            tc.tile_pool(name="mst", bufs=1) as mpool,
            tc.tile_pool(name="outp", bufs=1) as opool,
            tc.tile_pool(name="ps1", bufs=1, space="PSUM") as ps1,
            tc.tile_pool(name="ps2", bufs=1, space="PSUM") as ps2,
        ):
            # constants (on the scalar HWDGE queue so the sync queue starts
            # streaming feat/W immediately)
            mbdt_t = cpool.tile([P, KC1, S], cdt)
            nc.scalar.dma_start(mbdt_t[:], mbdt[:].rearrange("p (k j) -> p k j", k=KC1))
            ones_t = cpool.tile([1, S], cdt)
            nc.vector.memset(ones_t[:], 1.0)
            bias_t = cpool.tile([1, D], cdt)
            nc.scalar.dma_start(bias_t[:], bias[:])

            msT = mpool.tile([P, KC2, S], cdt)  # masked_sum transposed

            # Column-group tiling: the four 512-wide output slices live at
            # partition offsets 0/32/64/96 of ONE psum bank, so their four
            # matmuls (same stationary operand) run concurrently in four PE
            # column groups instead of serializing on the weight reload.
            pm_bank = ps1.tile([P, 512], F32)
            po_bank = ps2.tile([P, 512], F32)

            # ---- stage 1: masked_sum[j, d] = sum_(s,a) mbd[(s,a), j] feat[(s,a), d]
            for g in range(KC1 // cf):
                ft = fpool.tile([P, cf, D], cdt)
                nc.sync.dma_start(
                    ft[:],
                    feat[g * cf * P : (g + 1) * cf * P, :].rearrange(
                        "(c p) d -> p c d", p=P
                    ),
                )
                for c in range(cf):
                    k = g * cf + c
                    for n in range(NT):
                        nc.tensor.matmul(
                            pm_bank[n * S : (n + 1) * S, :],
                            mbdt_t[:, k, :],
                            ft[:, c, n * 512 : (n + 1) * 512],
                            start=(k == 0),
                            stop=(k == KC1 - 1),
                            tile_position=(0, n * S),
                            skip_group_check=True,
                        )
            # cast/copy psum -> sbuf, then 32x32 block transposes into msT
            msc = mcpool.tile([P, 512], cdt)
            nc.any.tensor_copy(msc[:], pm_bank[:])
            for n in range(NT):
                for q in range(512 // 32):
                    d0 = n * 512 + q * 32
                    k2, r = divmod(d0, P)
                    nc.vector.transpose(
                        msT[r : r + 32, k2, :],
                        msc[n * S : (n + 1) * S, q * 32 : (q + 1) * 32],
                    )

            # ---- stage 2: out = tanh(masked_sum @ W + b)
            for g in range(KC2 // cw):
                wt = wpool.tile([P, cw, D], cdt)
                nc.sync.dma_start(
                    wt[:],
                    w[g * cw * P : (g + 1) * cw * P, :].rearrange(
                        "(c p) d -> p c d", p=P
                    ),
                )
                for c in range(cw):
                    k2 = g * cw + c
                    for n in range(NT):
                        nc.tensor.matmul(
                            po_bank[n * S : (n + 1) * S, :],
                            msT[:, k2, :],
                            wt[:, c, n * 512 : (n + 1) * 512],
                            start=(k2 == 0),
                            stop=False,
                            tile_position=(0, n * S),
                            skip_group_check=True,
                        )
            for n in range(NT):
                # bias as rank-1 matmul: po[j, :] += ones[j] * b[:]
                nc.tensor.matmul(
                    po_bank[n * S : (n + 1) * S, :],
                    ones_t[:],
                    bias_t[:, n * 512 : (n + 1) * 512],
                    start=False,
                    stop=True,
                    tile_position=(0, n * S),
                    skip_group_check=True,
                )
            out_sb = opool.tile([P, 512], F32)
            nc.scalar.activation(
                out_sb[:], po_bank[:], mybir.ActivationFunctionType.Tanh
            ).then_inc(act_sem, 1)
            scalar.wait_ge(act_sem, 1)
            for n in range(NT):
                nc.scalar.dma_start(
                    out[:, n * 512 : (n + 1) * 512],
                    out_sb[n * S : (n + 1) * S, :],
                )
    nc.compile()
    return nc


def build_nc_raw(compute_dtype: str = COMPUTE_DTYPE):
    """Raw-bass (no Tile) version: manual semaphores, everything resident in
    SBUF (feat 32KB/part + W 64KB/part at fp16), minimal prologue/epilogue."""
    cdt = {"f32": mybir.dt.float32, "bf16": mybir.dt.bfloat16, "fp16": mybir.dt.float16}[compute_dtype]
    cf = 4
    WCH = [4, 4, 4, 4]  # uniform W transfer sizes (k2-chunks)
    WST = [0, 4, 8, 12]
    NF, NW = KC1 // cf, len(WCH)
    nc = bacc.Bacc("TRN2", target_bir_lowering=False, debug=False)

    feat = nc.dram_tensor("feat", [S * A, D], cdt, kind="ExternalInput")
    mbdt = nc.dram_tensor("mbdt", [P, KC1 * S], cdt, kind="ExternalInput")
    w = nc.dram_tensor("w", [D, D], cdt, kind="ExternalInput")
    bias = nc.dram_tensor("bias", [1, D], cdt, kind="ExternalInput")
    onesd = nc.dram_tensor("ones", [1, S], cdt, kind="ExternalInput")
    out = nc.dram_tensor("out", [S, D], F32, kind="ExternalOutput")

    from contextlib import ExitStack

    with ExitStack() as ctx:
        feat_sb = ctx.enter_context(nc.sbuf_tensor([P, KC1, D], cdt))
        w_sb = ctx.enter_context(nc.sbuf_tensor([P, KC2, D], cdt))
        mbdt_sb = ctx.enter_context(nc.sbuf_tensor([P, KC1, S], cdt))
        bias_sb = ctx.enter_context(nc.sbuf_tensor([1, D], cdt))
        ones_sb = ctx.enter_context(nc.sbuf_tensor([1, S], cdt))
        msc_sb = ctx.enter_context(nc.sbuf_tensor([P, 512], cdt))
        msT_sb = ctx.enter_context(nc.sbuf_tensor([P, KC2, S], cdt))
        out_sb = ctx.enter_context(nc.sbuf_tensor([P, 512], F32))
        pm_bank = ctx.enter_context(nc.psum_tensor([P, 512], F32))
        po_bank = ctx.enter_context(nc.psum_tensor([P, 512], F32))
        fsems = [ctx.enter_context(nc.semaphore(f"fs{g}")) for g in range(NF)]
        wsems = [ctx.enter_context(nc.semaphore(f"ws{g}")) for g in range(NW)]
        csem = ctx.enter_context(nc.semaphore("csem"))
        osem = ctx.enter_context(nc.semaphore("osem"))
        s1_sem = ctx.enter_context(nc.semaphore("s1_sem"))
        tr_sem = ctx.enter_context(nc.semaphore("tr_sem"))
        s2_sem = ctx.enter_context(nc.semaphore("s2_sem"))
        act_sem = ctx.enter_context(nc.semaphore("act_sem"))
        osem2 = ctx.enter_context(nc.semaphore("osem2"))
        block = ctx.enter_context(nc.Block(no_gpsimd_drain=True))

        @block.sync
        def _(sync):
            for g in range(NF):
                sync.dma_start(
                    feat_sb[:, g * cf : (g + 1) * cf, :],
                    feat[g * cf * P : (g + 1) * cf * P, :].rearrange(
                        "(c p) d -> p c d", p=P
                    ),
                ).then_inc(fsems[g], 16)
            for g in range(NW):
                st, ln = WST[g], WCH[g]
                sync.dma_start(
                    w_sb[:, st : st + ln, :],
                    w[st * P : (st + ln) * P, :].rearrange(
                        "(c p) d -> p c d", p=P
                    ),
                ).then_inc(wsems[g], 16)
            sync.wait_ge(act_sem, 1)
            for n in (0, 2):
                sync.dma_start(
                    out[:, n * 512 : (n + 1) * 512], out_sb[n * S : (n + 1) * S, :]
                ).then_inc(osem2, 16)
            sync.wait_ge(osem2, 32)

        @block.scalar
        def _(scalar):
            scalar.dma_start(
                mbdt_sb[:], mbdt[:].rearrange("p (k j) -> p k j", k=KC1)
            ).then_inc(csem, 16)
            scalar.dma_start(bias_sb[:], bias[:]).then_inc(csem, 16)
            scalar.dma_start(ones_sb[:], onesd[:]).then_inc(csem, 16)
            scalar.wait_ge(s2_sem, 1)
            nc.scalar.activation(
                out_sb[:], po_bank[:], mybir.ActivationFunctionType.Tanh
            ).then_inc(act_sem, 1)
            scalar.wait_ge(act_sem, 1)
            for n in (1, 3):
                scalar.dma_start(
                    out[:, n * 512 : (n + 1) * 512], out_sb[n * S : (n + 1) * S, :]
                ).then_inc(osem, 16)
            scalar.wait_ge(osem, 32)

        @block.vector
        def _(vector):
            vector.wait_ge(s1_sem, 1)
            nc.vector.tensor_copy(msc_sb[:], pm_bank[:])
            nc.vector.drain()
            last = None
            for n in range(NT):
                for q in range(512 // 32):
                    d0 = n * 512 + q * 32
                    k2, r = divmod(d0, P)
                    last = nc.vector.transpose(
                        msT_sb[r : r + 32, k2, :],
                        msc_sb[n * S : (n + 1) * S, q * 32 : (q + 1) * 32],
                    )
            last.then_inc(tr_sem, 1)

        @block.tensor
        def _(tensor):
            tensor.wait_ge(csem, 48)  # mbdt/bias/ones resident
            # bias as the FIRST accumulation into po_bank (off the tail path)
            for n in range(NT):
                nc.tensor.matmul(
                    po_bank[n * S : (n + 1) * S, :],
                    ones_sb[:],
                    bias_sb[:, n * 512 : (n + 1) * 512],
                    start=True,
                    stop=False,
                    tile_position=(0, n * S),
                    skip_group_check=True,
                )
            last = None
            for g in range(NF):
                tensor.wait_ge(fsems[g], 16)
                for c in range(cf):
                    k = g * cf + c
                    for n in range(NT):
                        last = nc.tensor.matmul(
                            pm_bank[n * S : (n + 1) * S, :],
                            mbdt_sb[:, k, :],
                            feat_sb[:, k, n * 512 : (n + 1) * 512],
                            start=(k == 0),
                            stop=(k == KC1 - 1),
                            tile_position=(0, n * S),
                            skip_group_check=True,
                        )
            last.then_inc(s1_sem, 1)
            tensor.wait_ge(tr_sem, 1)
            lastb = None
            for g in range(NW):
                tensor.wait_ge(wsems[g], 16)
                for c in range(WCH[g]):
                    k2 = WST[g] + c
                    for n in range(NT):
                        lastb = nc.tensor.matmul(
                            po_bank[n * S : (n + 1) * S, :],
                            msT_sb[:, k2, :],
                            w_sb[:, k2, n * 512 : (n + 1) * 512],
                            start=False,
                            stop=(k2 == KC2 - 1),
                            tile_position=(0, n * S),
                            skip_group_check=True,
                        )
            lastb.then_inc(s2_sem, 1)

    nc.compile()
    return nc


def _host_prep(inputs: dict, compute_dtype: str):
    np_cdt = {"f32": np.float32, "bf16": None, "fp16": np.float16}[compute_dtype]
    if np_cdt is None:
        import ml_dtypes

        np_cdt = ml_dtypes.bfloat16

    feat = np.ascontiguousarray(inputs["attribute_feat"], dtype=np.float32)
    label = np.asarray(inputs["attribute_label"])
    w = np.asarray(inputs["W"], dtype=np.float32).astype(np_cdt)
    b = np.asarray(inputs["b"], dtype=np.float32).reshape(1, D).astype(np_cdt)
    mask = (label > 0).astype(np.float32)

    in_maps = []
    for c in range(NCORES):
        feat_c = feat[c * S : (c + 1) * S].reshape(S * A, D).astype(np_cdt)
        m_c = mask[c * S : (c + 1) * S]  # [S, A]
        mbd = np.zeros((KC1, P, S), np.float32)
        for k in range(KC1):
            for sl in range(P // A):  # 4 samples per 128-row chunk
                s = (P // A) * k + sl
                mbd[k, sl * A : (sl + 1) * A, s] = m_c[s]
        # device layout: [partition, (k_chunk, sample)] contiguous
        mbd_dev = np.ascontiguousarray(mbd.transpose(1, 0, 2)).reshape(P, KC1 * S)
        in_maps.append(
            {
                "feat": feat_c,
                "mbdt": mbd_dev.astype(np_cdt),
                "w": w,
                "bias": b,
                "ones": np.ones((1, S), np_cdt),
            }
        )
    return in_maps


_NC_CACHE: dict = {}


def run(inputs: dict, compute_dtype: str = COMPUTE_DTYPE, trace: bool = False):
    from concourse.bass_utils import run_bass_kernel_spmd

    impl = os.environ.get("GNN_KERNEL_IMPL", "raw")
    key = (compute_dtype, impl)
    if key not in _NC_CACHE:
        builder = build_nc_raw if impl == "raw" else build_nc
        _NC_CACHE[key] = builder(compute_dtype)
    nc = _NC_CACHE[key]
    in_maps = _host_prep(inputs, compute_dtype)
    res = run_bass_kernel_spmd(nc, in_maps, list(range(NCORES)), trace=trace)
    out = np.concatenate([res.results[c]["out"] for c in range(NCORES)], axis=0)
    return out, res


def kernel(**inputs) -> np.ndarray:
    out, _ = run(inputs)
    return out

